# revision 2
# baseline (speedup 1.0000x reference)
"""Trainium2 Bass kernel for nn_AdaptiveSlotAttention (8 NeuronCores, data-parallel over batch).

Device (per core, 4 of 32 batches):
  NEFF A: X = features @ w_in + b_in -> LayerNorm(ln1) -> LayerNorm(ninput) = inp
  NEFF B: k = inp @ wk, v = inp @ wv
These two phases are ~87% of total FLOPs. The small slot-iteration phase
([384, 512] GRU/FFN/attention, ~11 GFLOP) runs vectorized on host in fp32.
"""
import sys

sys.path.insert(0, "/opt/trn_rl_repo")

import numpy as np

import concourse.bass as bass
import concourse.mybir as mybir
import concourse.tile as tile
from concourse import bacc
from concourse.bass_utils import run_bass_kernel_spmd

B, N, F, D, H, S_OBJ, S_REG, ITERS = 32, 1024, 1152, 512, 8, 8, 4, 3
S = S_OBJ + S_REG
HD = D // H
SCALE = HD ** -0.5
NC = 8
BL = B // NC          # batches per core
R = BL * N            # rows per core (4096)
P = 128
MM_DT = mybir.dt.float32

_CACHE = {}


def _bcast_ap(dram_ap, parts=P):
    """AP that broadcasts a 1-D DRAM vector across `parts` partitions."""
    return bass.AP(
        tensor=dram_ap.tensor,
        offset=dram_ap.offset,
        ap=[[0, parts]] + [list(x) for x in dram_ap.ap],
    )


def _build_neff_a():
    nc = bacc.Bacc("TRN2", target_bir_lowering=False, debug=False,
                   enable_asserts=False, num_devices=NC)
    featT = nc.dram_tensor("featT", [F, R], mybir.dt.float32, kind="ExternalInput").ap()
    w_in = nc.dram_tensor("w_in", [F, D], mybir.dt.float32, kind="ExternalInput").ap()
    b_in = nc.dram_tensor("b_in", [D], mybir.dt.float32, kind="ExternalInput").ap()
    g1 = nc.dram_tensor("g1", [D], mybir.dt.float32, kind="ExternalInput").ap()
    b1 = nc.dram_tensor("b1", [D], mybir.dt.float32, kind="ExternalInput").ap()
    g2 = nc.dram_tensor("g2", [D], mybir.dt.float32, kind="ExternalInput").ap()
    b2 = nc.dram_tensor("b2", [D], mybir.dt.float32, kind="ExternalInput").ap()
    inp_out = nc.dram_tensor("inp", [R, D], mybir.dt.float32, kind="ExternalOutput").ap()

    KO = F // P  # 9
    featT_t = featT.rearrange("(ko ki) m -> ki ko m", ki=P)

    with tile.TileContext(nc) as tc:
        with (
            tc.tile_pool(name="singles", bufs=1) as singles,
            tc.tile_pool(name="lhs", bufs=3) as lhs_pool,
            tc.tile_pool(name="work", bufs=3) as work,
            tc.tile_pool(name="stats", bufs=4) as stats,
            tc.tile_pool(name="psum", bufs=2, space="PSUM") as psum_pool,
        ):
            w_t = singles.tile([P, KO, D], mybir.dt.float32)
            nc.sync.dma_start(w_t[:], w_in.rearrange("(ko ki) d -> ki ko d", ki=P))
            bias_t = singles.tile([P, D], mybir.dt.float32)
            nc.gpsimd.dma_start(bias_t[:], _bcast_ap(b_in))
            g1_t = singles.tile([P, D], mybir.dt.float32)
            nc.gpsimd.dma_start(g1_t[:], _bcast_ap(g1))
            b1_t = singles.tile([P, D], mybir.dt.float32)
            nc.gpsimd.dma_start(b1_t[:], _bcast_ap(b1))
            g2_t = singles.tile([P, D], mybir.dt.float32)
            nc.gpsimd.dma_start(g2_t[:], _bcast_ap(g2))
            b2_t = singles.tile([P, D], mybir.dt.float32)
            nc.gpsimd.dma_start(b2_t[:], _bcast_ap(b2))
            eps_t = singles.tile([P, 1], mybir.dt.float32)
            nc.vector.memset(eps_t, 1e-5)

            for m in range(R // P):
                ft = lhs_pool.tile([P, KO, P], mybir.dt.float32)
                nc.sync.dma_start(ft[:], featT_t[:, :, m * P:(m + 1) * P])
                ps = psum_pool.tile([P, D], mybir.dt.float32)
                for ko in range(KO):
                    nc.tensor.matmul(
                        ps[:],
                        lhsT=ft[:, ko, :].bitcast(MM_DT),
                        rhs=w_t[:, ko, :].bitcast(MM_DT),
                        start=(ko == 0),
                        stop=(ko == KO - 1),
                    )
                x_t = work.tile([P, D], mybir.dt.float32)
                nc.vector.tensor_add(x_t[:], ps[:], bias_t[:])

                for (g_t, b_t) in ((g1_t, b1_t), (g2_t, b2_t)):
                    st = stats.tile([P, nc.vector.BN_STATS_DIM], mybir.dt.float32)
                    nc.vector.bn_stats(out=st[:], in_=x_t[:])
                    mv = stats.tile([P, nc.vector.BN_AGGR_DIM], mybir.dt.float32)
                    nc.vector.bn_aggr(out=mv[:], in_=st[:])
                    nc.scalar.activation(
                        out=mv[:, 1:2], in_=mv[:, 1:2],
                        func=mybir.ActivationFunctionType.Sqrt,
                        bias=eps_t[:], scale=1.0, alpha=0.0,
                    )
                    nc.vector.reciprocal(out=mv[:, 1:2], in_=mv[:, 1:2])
                    nc.vector.tensor_scalar(
                        out=x_t[:], in0=x_t[:],
                        scalar1=mv[:, 0:1], scalar2=mv[:, 1:2],
                        op0=mybir.AluOpType.subtract, op1=mybir.AluOpType.mult,
                    )
                    nc.vector.tensor_mul(x_t[:], x_t[:], g_t[:])
                    nc.vector.tensor_add(x_t[:], x_t[:], b_t[:])

                nc.sync.dma_start(inp_out[m * P:(m + 1) * P, :], x_t[:])
    nc.compile()
    return nc


def _build_neff_b():
    nc = bacc.Bacc("TRN2", target_bir_lowering=False, debug=False,
                   enable_asserts=False, num_devices=NC)
    inpT = nc.dram_tensor("inpT", [D, R], mybir.dt.float32, kind="ExternalInput").ap()
    wk = nc.dram_tensor("wk", [D, D], mybir.dt.float32, kind="ExternalInput").ap()
    wv = nc.dram_tensor("wv", [D, D], mybir.dt.float32, kind="ExternalInput").ap()
    k_out = nc.dram_tensor("k", [R, D], mybir.dt.float32, kind="ExternalOutput").ap()
    v_out = nc.dram_tensor("v", [R, D], mybir.dt.float32, kind="ExternalOutput").ap()

    KO = D // P  # 4
    inpT_t = inpT.rearrange("(ko ki) m -> ki ko m", ki=P)

    with tile.TileContext(nc) as tc:
        with (
            tc.tile_pool(name="singles", bufs=1) as singles,
            tc.tile_pool(name="lhs", bufs=3) as lhs_pool,
            tc.tile_pool(name="work", bufs=4) as work,
            tc.tile_pool(name="psum", bufs=2, space="PSUM") as psum_pool,
        ):
            wk_t = singles.tile([P, KO, D], mybir.dt.float32)
            nc.sync.dma_start(wk_t[:], wk.rearrange("(ko ki) d -> ki ko d", ki=P))
            wv_t = singles.tile([P, KO, D], mybir.dt.float32)
            nc.sync.dma_start(wv_t[:], wv.rearrange("(ko ki) d -> ki ko d", ki=P))

            for m in range(R // P):
                it = lhs_pool.tile([P, KO, P], mybir.dt.float32)
                nc.sync.dma_start(it[:], inpT_t[:, :, m * P:(m + 1) * P])
                for (w_t, o_ap) in ((wk_t, k_out), (wv_t, v_out)):
                    ps = psum_pool.tile([P, D], mybir.dt.float32)
                    for ko in range(KO):
                        nc.tensor.matmul(
                            ps[:],
                            lhsT=it[:, ko, :].bitcast(MM_DT),
                            rhs=w_t[:, ko, :].bitcast(MM_DT),
                            start=(ko == 0),
                            stop=(ko == KO - 1),
                        )
                    o_t = work.tile([P, D], mybir.dt.float32)
                    nc.vector.tensor_copy(o_t[:], ps[:])
                    nc.sync.dma_start(o_ap[m * P:(m + 1) * P, :], o_t[:])
    nc.compile()
    return nc


def _get_neffs():
    if "a" not in _CACHE:
        _CACHE["a"] = _build_neff_a()
        _CACHE["b"] = _build_neff_b()
    return _CACHE["a"], _CACHE["b"]


def _erf(x):
    try:
        from scipy.special import erf
        return erf(x).astype(np.float32)
    except ImportError:
        import math
        v = np.vectorize(math.erf)
        return v(x).astype(np.float32)


def kernel(**inputs):
    g = {k: np.asarray(v, dtype=np.float32) for k, v in inputs.items()}
    feats = g["features"]
    noise = g["noise"]
    nc_a, nc_b = _get_neffs()

    in_maps_a = []
    for c in range(NC):
        sh = feats[c * BL:(c + 1) * BL].reshape(R, F)
        in_maps_a.append({
            "featT": np.ascontiguousarray(sh.T),
            "w_in": g["w_in"], "b_in": g["b_in"],
            "g1": g["ln1_g"], "b1": g["ln1_b"],
            "g2": g["ninput_g"], "b2": g["ninput_b"],
        })
    res_a = run_bass_kernel_spmd(nc_a, in_maps_a, core_ids=list(range(NC)))
    inp_shards = [res_a.results[c]["inp"] for c in range(NC)]

    in_maps_b = []
    for c in range(NC):
        in_maps_b.append({
            "inpT": np.ascontiguousarray(inp_shards[c].T),
            "wk": g["wk"], "wv": g["wv"],
        })
    res_b = run_bass_kernel_spmd(nc_b, in_maps_b, core_ids=list(range(NC)))
    k = np.concatenate([res_b.results[c]["k"] for c in range(NC)], 0).reshape(B, N, H, HD)
    v = np.concatenate([res_b.results[c]["v"] for c in range(NC)], 0).reshape(B, N, H, HD)

    # --- host iteration phase (fp32, vectorized over full batch) ---
    def ln(x, gg, bb, eps=np.float32(1e-5)):
        m = x.mean(-1, keepdims=True, dtype=np.float32)
        vv = ((x - m) ** 2).mean(-1, keepdims=True, dtype=np.float32)
        return (x - m) / np.sqrt(vv + eps) * gg + bb

    slots = (g["slot_mu"] + np.exp(g["slot_log_sigma"]) * noise).astype(np.float32)
    attn_out = None
    for _ in range(ITERS):
        sn = ln(slots, g["nslots_g"], g["nslots_b"])
        q = (sn.reshape(B * S, D) @ g["wq"]).reshape(B, S, H, HD)
        dots = np.einsum("bihd,bjhd->bihj", q, k).astype(np.float32) * np.float32(SCALE)
        m = dots.reshape(B, S * H, N)
        e = np.exp(m - m.max(1, keepdims=True))
        attn = (e / e.sum(1, keepdims=True)).reshape(B, S, H, N)
        attn_out = attn.mean(2)
        attn_n = (attn + np.float32(1e-8)) / (attn.sum(-1, keepdims=True) + np.float32(1e-8))
        upd = np.einsum("bjhd,bihj->bihd", v, attn_n).astype(np.float32).reshape(B * S, D)
        h = slots.reshape(B * S, D)
        gx = upd @ g["gru_wih"] + g["gru_bih"]
        gh = h @ g["gru_whh"] + g["gru_bhh"]
        xr, xz, xn = np.split(gx, 3, -1)
        hr, hz, hn = np.split(gh, 3, -1)
        r = 1.0 / (1.0 + np.exp(-(xr + hr)))
        z = 1.0 / (1.0 + np.exp(-(xz + hz)))
        n = np.tanh(xn + r * hn)
        slots = ((1.0 - z) * n + z * h).astype(np.float32).reshape(B, S, D)
        f = ln(slots, g["ffn_ln_g"], g["ffn_ln_b"])
        f1 = f.reshape(B * S, D) @ g["ffn_w1"] + g["ffn_b1"]
        f1 = (0.5 * f1 * (1.0 + _erf(f1 / np.sqrt(np.float32(2.0))))).astype(np.float32)
        f2 = f1 @ g["ffn_w2"] + g["ffn_b2"]
        slots = slots + f2.reshape(B, S, D)

    obj = slots[:, :S_OBJ]
    lg = np.maximum(obj.reshape(-1, D) @ g["sel_w1"] + g["sel_b1"], 0.0)
    logits = (lg @ g["sel_w2"] + g["sel_b2"]).reshape(B, S_OBJ, 2)
    decision = (np.argmax(logits, -1) == 1).astype(np.float32)
    needed = np.maximum(0.0, 2.0 - decision.sum(-1, keepdims=True))
    rank = np.cumsum(1.0 - decision, -1)
    add = ((decision == 0) & (rank <= needed)).astype(np.float32)
    decision = decision + add
    obj = (obj * decision[..., None]).astype(np.float32)
    return obj, decision.astype(np.float32), attn_out[:, :S_OBJ].astype(np.float32)
